# revision 1
# baseline (speedup 1.0000x reference)
"""Distributed Trainium2 (Bass/Tile) kernel for the contrastive loss.

Strategy (8 NeuronCores, SPMD, row-sharded similarity matrix):
  Core c owns 512 of the 4096 rows of sim = reps @ reps^T (per l).
  The host rolls the column order by c*512 for each core so a single
  NEFF serves all cores: the self-match column for local row r is
  always column r, and the positive-pair column is always column
  r + 2048.  Each core:
    - loads all 4096 raw embedding rows (natural [row, d] layout),
    - L2-normalizes rows (fused square+row-sum on DVE; inv-norm via
      exp(-0.5*ln(ssq)) so every ACT op stays in one function-table set),
    - transposes normalized rows to [d, row] via PE matmuls vs identity,
    - computes its 512x4096 row-block of sim on PE (K=D=128 single shot),
    - exp(sim/T) + row-sum fused on ACT (activation accum_out),
    - extracts self/positive diagonal entries via masked
      tensor_tensor_reduce against identity on DVE,
    - combines into per-row weighted loss terms, DMAs out [128, 16].
  Host sums the 8 partial tensors -> scalar loss (the all-reduce).
"""

import numpy as np

TEMP = 0.2
L, B, K, D = 4, 64, 32, 128
N = B * K          # 2048
M = 2 * N          # 4096 rows of sim per l
NCORES = 8
R = M // NCORES    # 512 local rows per core
SEG = M // 128     # 32 row-tiles of 128 per l
INV_T = 1.0 / TEMP

_built = None


def _build():
    global _built
    if _built is not None:
        return _built
    from contextlib import ExitStack

    import concourse.tile as tile
    from concourse import bacc
    import concourse.mybir as mybir
    from concourse.masks import make_identity

    f32 = mybir.dt.float32
    AF = mybir.ActivationFunctionType
    OP = mybir.AluOpType
    AX = mybir.AxisListType

    # Pin every ACT op to the natural_log_exp_and_others table set (it covers
    # Copy/Exp/Identity/Ln/Square — everything we use), so bacc emits exactly
    # one LoadActFuncSet instead of thrashing ~2.7us loads between sets.
    # Other sets are EMPTIED (not removed) so act_func_set_id indices into
    # act_info.json stay valid.
    from concourse import hw_specs as _hw
    _tabs = dict(_hw.get_activation_tables("gen3"))
    _pinned = {
        name: (fns if name == "natural_log_exp_and_others" else frozenset())
        for name, fns in _tabs.items()
    }
    _hw.get_activation_tables.cache_clear()
    _orig = _hw.get_activation_tables.__wrapped__

    def _patched(arch):
        if arch == "gen3":
            return _pinned
        return _orig(arch)

    _hw.get_activation_tables = _patched
    import concourse.bacc as _baccmod
    if hasattr(_baccmod, "get_activation_tables"):
        _baccmod.get_activation_tables = _patched

    nc = bacc.Bacc(None, target_bir_lowering=False)
    emb = nc.dram_tensor("emb_nat", [128, L, SEG, D], f32, kind="ExternalInput")
    jvl = nc.dram_tensor("jv_local", [R], f32, kind="ExternalInput")
    out = nc.dram_tensor("out_wlp", [128, 4 * L], f32, kind="ExternalOutput")

    with ExitStack() as ctx:
        tc = ctx.enter_context(tile.TileContext(nc))
        singles = ctx.enter_context(tc.tile_pool(name="singles", bufs=1))
        natp = ctx.enter_context(tc.tile_pool(name="nat", bufs=8))
        xtp = ctx.enter_context(tc.tile_pool(name="xt", bufs=16))
        junkp = ctx.enter_context(tc.tile_pool(name="junk", bufs=4))
        statp = ctx.enter_context(tc.tile_pool(name="stat", bufs=2))
        expp = ctx.enter_context(tc.tile_pool(name="expo", bufs=4))
        tpp = ctx.enter_context(tc.tile_pool(name="tp", bufs=2, space="PSUM"))
        simp = ctx.enter_context(tc.tile_pool(name="sim", bufs=3, space="PSUM"))

        ident = singles.tile([128, 128], f32)
        make_identity(nc, ident[:])

        w = singles.tile([128, 4], f32)
        nc.sync.dma_start(out=w[:], in_=jvl.rearrange("(rb p) -> p rb", p=128))

        dsum = singles.tile([128, 4 * L, 4], f32)  # per (l,rb): 4 chunk sums
        selfb = singles.tile([128, 4 * L], f32)
        posb = singles.tile([128, 4 * L], f32)

        for l in range(L):
            # one contiguous DMA per l (128 x 16KB descriptors); issued from
            # the otherwise-idle GpSimd queue to keep SP free
            nat = natp.tile([128, SEG, D], f32)
            nc.gpsimd.dma_start(out=nat[:], in_=emb[:, l, :, :])

            ssq = statp.tile([128, SEG], f32)
            lnssq = statp.tile([128, SEG], f32)
            invn = statp.tile([128, SEG], f32)
            for g2 in range(4):
                for s8 in range(8):
                    s = g2 * 8 + s8
                    junk = junkp.tile([128, D], f32)
                    nc.vector.scalar_tensor_tensor(
                        out=junk[:], in0=nat[:, s, :], scalar=1.0,
                        in1=nat[:, s, :],
                        op0=OP.mult, op1=OP.mult, accum_out=ssq[:, s : s + 1])
                # inv_norm = exp(-0.5*ln(ssq)); Ln+Exp share one ACT table set
                sl = slice(g2 * 8, (g2 + 1) * 8)
                nc.scalar.activation(out=lnssq[:, sl], in_=ssq[:, sl], func=AF.Ln)
                nc.scalar.activation(
                    out=invn[:, sl], in_=lnssq[:, sl], func=AF.Exp, scale=-0.5)
                for s8 in range(8):
                    s = g2 * 8 + s8
                    nc.vector.tensor_scalar_mul(
                        nat[:, s, :], nat[:, s, :], invn[:, s : s + 1])

            # transpose normalized rows into [d, row] chunks of 512 columns
            xtc = []
            for g in range(8):
                ps = tpp.tile([128, 512], f32)
                for kk in range(4):
                    s = g * 4 + kk
                    nc.tensor.matmul(
                        ps[:, kk * 128 : (kk + 1) * 128], nat[:, s, :], ident[:],
                        start=True, stop=True)
                xc = xtp.tile([128, 512], f32)
                nc.vector.tensor_copy(xc[:], ps[:])
                xtc.append(xc)

            # the 512x4096 sim row-block for this l
            for rb in range(4):
                lr = l * 4 + rb
                lhsT = xtc[0][:, rb * 128 : (rb + 1) * 128]
                for t in range(4):
                    sim = simp.tile([128, 1024], f32)
                    for u in range(2):
                        fc = t * 2 + u
                        nc.tensor.matmul(
                            sim[:, u * 512 : (u + 1) * 512], lhsT,
                            xtc[fc][:],
                            start=True, stop=True)
                    eo = expp.tile([128, 1024], f32)
                    nc.scalar.activation(
                        out=eo[:], in_=sim[:], func=AF.Exp, scale=INV_T,
                        accum_out=dsum[:, lr, t : t + 1])
                    if t == 0 or t == 2:
                        # self-sim diagonal (t=0) / positive-pair diagonal (t=2)
                        buf = selfb if t == 0 else posb
                        junk = junkp.tile([128, 128], f32)
                        nc.vector.scalar_tensor_tensor(
                            out=junk[:], in0=sim[:, rb * 128 : rb * 128 + 128],
                            scalar=1.0, in1=ident[:],
                            op0=OP.mult, op1=OP.mult,
                            accum_out=buf[:, lr : lr + 1])

        # tail: per-row loss terms
        denom = singles.tile([128, 4 * L], f32)
        nc.vector.reduce_sum(out=denom[:], in_=dsum[:], axis=AX.X)
        selfexp = singles.tile([128, 4 * L], f32)
        nc.scalar.activation(out=selfexp[:], in_=selfb[:], func=AF.Exp, scale=INV_T)
        nc.vector.tensor_sub(denom[:], denom[:], selfexp[:])
        logd = singles.tile([128, 4 * L], f32)
        nc.scalar.activation(out=logd[:], in_=denom[:], func=AF.Ln)
        lp = singles.tile([128, 4 * L], f32)
        nc.vector.tensor_scalar_mul(lp[:], posb[:], -INV_T)
        nc.vector.tensor_add(lp[:], lp[:], logd[:])
        wlp = singles.tile([128, 4 * L], f32)
        for l in range(L):
            nc.vector.tensor_mul(
                wlp[:, l * 4 : (l + 1) * 4], lp[:, l * 4 : (l + 1) * 4], w[:])
        nc.sync.dma_start(out=out[:, :], in_=wlp[:])

    nc.finalize()
    _built = nc
    return nc


def _in_maps(emb_i, emb_j, joint_valid):
    emb_i = np.asarray(emb_i, dtype=np.float32)
    emb_j = np.asarray(emb_j, dtype=np.float32)
    jv = np.asarray(joint_valid, dtype=np.float32).reshape(-1)
    reps = np.concatenate(
        [emb_i.reshape(L, N, D), emb_j.reshape(L, N, D)], axis=1)  # [L, M, D]
    maps = []
    for c in range(NCORES):
        idx = (np.arange(M) + c * R) % M
        cols = reps[:, idx, :]  # rolled so local rows sit at columns 0..R-1
        nat = np.ascontiguousarray(
            cols.reshape(L, SEG, 128, D).transpose(2, 0, 1, 3))
        jvl = np.ascontiguousarray(jv[(np.arange(R) + c * R) % N])
        maps.append({"emb_nat": nat, "jv_local": jvl})
    return maps, jv


def _combine(results, jv):
    tot = 0.0
    for r in results:
        tot += float(r["out_wlp"].astype(np.float64).sum())
    return np.float32(tot / (2.0 * float(jv.sum())))


def kernel(emb_i, emb_j, joint_valid):
    from concourse.bass_utils import run_bass_kernel_spmd

    nc = _build()
    maps, jv = _in_maps(emb_i, emb_j, joint_valid)
    res = run_bass_kernel_spmd(nc, maps, core_ids=list(range(NCORES)))
    return _combine(res.results, jv)


def run_traced(inputs, trace_cores=None):
    """test.py helper: same run but with NTFF tracing enabled."""
    from concourse.bass_utils import run_bass_kernel_spmd

    nc = _build()
    maps, jv = _in_maps(**inputs)
    res = run_bass_kernel_spmd(
        nc, maps, core_ids=list(range(NCORES)), trace=True,
        trace_cores=trace_cores if trace_cores is not None else list(range(NCORES)))
    res.loss = _combine(res.results, jv)
    return res



# revision 15
# speedup vs baseline: 2.8838x; 2.8838x over previous
"""Distributed Trainium2 (Bass/Tile) kernel for the contrastive loss.

Strategy (8 NeuronCores, SPMD, row-sharded similarity matrix):
  Core c owns 512 of the 4096 rows of sim = reps @ reps^T (per l).
  Host rolls the column order by c*512 so one NEFF serves all cores.

  Data layout trick: host feeds the raw embeddings already TRANSPOSED
  ([D, row] bf16), so no on-device transpose of the big matrix is
  needed.  Sim tiles are computed "transposed": out[n, m] = x_n . z_m
  with n = all 4096 rows on partitions (raw, unnormalized) and
  m = 512 local rows (normalized) on the free axis.  The row
  normalization of the n side folds into the exp as a per-partition
  scale (ACT) or per-partition pow base (DVE/Pool):
      exp(sim[n,m]/T) = exp((inv_n/T) * G[n,m]) = (e^{inv_n/T})^G[n,m]
  where G = x_n . z_m and inv_n = 1/|x_n|.

  The exp work is split across THREE engines (ACT Exp / DVE pow /
  GpSimd pow) writing bf16 E tiles; per-row denominators are then
  column sums of E done with near-free stationary matmuls
  (lhsT = E chunk [128,128], rhs = ones -> [128,1], ap_size=1),
  accumulated in PSUM across all 32 n-chunks.

  The self-similarity column (huge: exp(|x_n|/T) ~ e^56) is clamped to
  exactly 16384 in the E tile and subtracted exactly via the Ln bias.
  The positive-pair value is recovered per-row from ln(E_diag)*inv_m.

  Norms: ssq via DVE square (tensor_scalar pow 2, 4x mode) + 128
  stationary matmuls against ones; inv = exp(-0.5 ln(ssq)) on ACT.

  Host sums the 8 output tensors -> scalar loss (the all-reduce).
"""

import numpy as np

TEMP = 0.2
INV_T = 1.0 / TEMP
L, B, K, D = 4, 64, 32, 128
N = B * K          # 2048
M = 2 * N          # 4096 rows of sim per l
NCORES = 8
R = M // NCORES    # 512 local rows per core
MS = 4             # m sub-blocks of 128
NCH = 32           # n-chunks of 128 per l
NT = 16            # sim tiles of [128, 2, 512] per l (2 chunks each)
CLAMP = 16384.0    # exactly representable in bf16; >> e^2.7, << e^40

# Exp engines (real-HW legal set): ACT Exp reads PSUM directly; DVE has no
# pow/exp, so DVE chunks use the Schraudolph float-bits trick -- one
# tensor_scalar (mult by per-partition A_n = inv_n/T * 2^23/ln2, add B)
# writing int32; the int bits ARE the bf-pattern of exp. GPSIMD cannot read
# PSUM at all, so Pool only gets SBUF-side scalar work.
SCH_S = 12102203.161561485          # 2^23 / ln 2
SCH_B = 1064868216.0                # 127*2^23 - 485000 (calibrated)

def _mk_sched():
    forced_a = {0, 1, 2, 3, 16, 17, 18, 19}
    pat = ["D", "D", "A", "D", "A", "D", "D", "A"]
    items, k = [], 0
    for j in range(32):
        if j in forced_a:
            items.append(("A", j))
        else:
            items.append((pat[k % 8], j)); k += 1
    return items

SCHED = _mk_sched()

_built = None


def _build():
    global _built
    if _built is not None:
        return _built
    from contextlib import ExitStack

    import concourse.tile as tile
    from concourse import bacc
    import concourse.mybir as mybir
    from concourse.masks import make_identity

    f32 = mybir.dt.float32
    bf16 = mybir.dt.bfloat16
    i32 = mybir.dt.int32
    AF = mybir.ActivationFunctionType
    OP = mybir.AluOpType

    # Pin every ACT op to the natural_log_exp_and_others table set (covers
    # Exp/Ln/Copy/Identity) so exactly one LoadActFuncSet is emitted.
    from concourse import hw_specs as _hw
    _tabs = dict(_hw.get_activation_tables("gen3"))
    _pinned = {
        name: (fns if name == "natural_log_exp_and_others" else frozenset())
        for name, fns in _tabs.items()
    }
    _hw.get_activation_tables.cache_clear()
    _orig = _hw.get_activation_tables.__wrapped__

    def _patched(arch):
        if arch == "gen3":
            return _pinned
        return _orig(arch)

    _hw.get_activation_tables = _patched
    import concourse.bacc as _baccmod
    if hasattr(_baccmod, "get_activation_tables"):
        _baccmod.get_activation_tables = _patched

    nc = bacc.Bacc(None, target_bir_lowering=False)
    xt = nc.dram_tensor("xt", [128, L, M], bf16, kind="ExternalInput")
    xloc = nc.dram_tensor("xloc", [128, L, MS, D], bf16, kind="ExternalInput")
    wv = nc.dram_tensor("wv", [128, MS], f32, kind="ExternalInput")
    out = nc.dram_tensor("out_wlp", [128, L * MS], f32, kind="ExternalOutput")
    import os
    _dbg = os.environ.get("KDBG", "0") == "1"
    if _dbg:
        dden = nc.dram_tensor("dbg_den", [128, L * MS], f32, kind="ExternalOutput")
        dpos = nc.dram_tensor("dbg_pos", [128, L, MS], f32, kind="ExternalOutput")

    with ExitStack() as ctx:
        tc = ctx.enter_context(tile.TileContext(nc))
        singles = ctx.enter_context(tc.tile_pool(name="singles", bufs=1))
        xtp = ctx.enter_context(tc.tile_pool(name="xtp", bufs=1))
        x2p = ctx.enter_context(tc.tile_pool(name="x2p", bufs=2))
        zp = ctx.enter_context(tc.tile_pool(name="zp", bufs=2))
        ep = ctx.enter_context(tc.tile_pool(name="ep", bufs=10))
        # PSUM: sim tiles 2 banks x3 bufs + ztp 1 bank + persist 1 bank = 8
        simp = ctx.enter_context(tc.tile_pool(name="simp", bufs=6, space="PSUM"))
        ztpp = ctx.enter_context(tc.tile_pool(name="ztpp", bufs=1, space="PSUM"))
        perp = ctx.enter_context(tc.tile_pool(name="perp", bufs=1, space="PSUM"))

        identb = singles.tile([128, 128], bf16)
        make_identity(nc, identb[:])
        ones = singles.tile([128, 1], bf16)
        nc.vector.memset(ones[:], 1.0)
        onesf = singles.tile([128, 1], f32)
        nc.vector.memset(onesf[:], 1.0)

        w = singles.tile([128, MS], f32)
        nc.sync.dma_start(out=w[:], in_=wv[:, :])
        negclamp = singles.tile([128, 1], f32)
        nc.vector.memset(negclamp[:], -CLAMP)

        # persistent PSUM bank: ssq (cols 0..127, [l*32+j]) and den
        # (cols 128..143, [l*4+s])
        persist = perp.tile([128, 512], f32)

        XT = xtp.tile([128, L, M], bf16)
        for l in range(L):
            nc.gpsimd.dma_start(out=XT[:, l, :], in_=xt[:, l, :])
        XL = singles.tile([128, L, MS, D], bf16)
        nc.sync.dma_start(out=XL[:], in_=xloc[:, :, :, :])

        # ---- per-l prologue: norms and normalized local rows ----
        def prologue(l):
            x2 = x2p.tile([128, M], bf16)
            nc.vector.tensor_scalar(out=x2[:], in0=XT[:, l, :], scalar1=2.0,
                                    scalar2=None, op0=OP.pow)
            ssq = persist[:, l * NCH:(l + 1) * NCH]
            for j in range(NCH):
                nc.tensor.matmul(ssq[:, j:j + 1], x2[:, j * 128:(j + 1) * 128],
                                 ones[:], start=True, stop=True)
            lnssq = singles.tile([128, NCH], f32)
            nc.scalar.activation(out=lnssq[:], in_=ssq, func=AF.Ln)
            inv = singles.tile([128, NCH], f32)
            nc.scalar.activation(out=inv[:], in_=lnssq[:], func=AF.Exp,
                                 scale=-0.5)
            invT = singles.tile([128, NCH], f32)
            nc.vector.tensor_scalar(out=invT[:], in0=inv[:], scalar1=INV_T,
                                    scalar2=None, op0=OP.mult)
            base = singles.tile([128, NCH], f32)
            nc.scalar.activation(out=base[:], in_=inv[:], func=AF.Exp,
                                 scale=INV_T)
            # normalized local rows -> [D, 512] via PE transpose
            zs = zp.tile([128, MS, D], bf16)
            for s in range(MS):
                nc.vector.tensor_scalar(out=zs[:, s, :], in0=XL[:, l, s, :],
                                        scalar1=inv[:, s:s + 1], scalar2=None,
                                        op0=OP.mult)
            ztp = ztpp.tile([128, R], bf16)
            for s in range(MS):
                nc.tensor.transpose(ztp[:, s * 128:(s + 1) * 128],
                                    zs[:, s, :], identb[:])
            zT = zp.tile([128, R], bf16)
            nc.vector.tensor_copy(zT[:], ztp[:])
            invs.append(inv)
            invTs.append(invT)
            bases.append(base)
            zTs.append(zT)

        posE = singles.tile([128, L, MS], f32)

        # ---- main loop: sim -> exp (ACT/DVE) -> colsum ----
        # deferred colsums so PE lags the exp engines by a few items
        pend = []   # (l, [j...], E_tile)

        def flush_colsum(item):
            l, j, E, kind = item
            den = denps[:, l * MS:(l + 1) * MS]
            for s in range(MS):
                lhs = E[:, s * 128:(s + 1) * 128]
                if kind == "D":
                    lhs = lhs.bitcast(f32)
                nc.tensor.matmul(
                    den[:, s:s + 1], lhs,
                    onesf[:] if kind == "D" else ones[:],
                    start=(j == 0), stop=(j == NCH - 1),
                    skip_group_check=True)
            if j == NCH - 1:
                tail_l(l)

        def extras(l, j, E):
            # self/pos chunks are forced to ACT, so E is bf16 here
            if j < MS:
                # clamp the self-similarity diagonal sub-square
                cols = slice(j * 128, (j + 1) * 128)
                nc.gpsimd.tensor_scalar(
                    out=E[:, cols], in0=E[:, cols], scalar1=CLAMP,
                    scalar2=None, op0=OP.min)
            if 16 <= j < 16 + MS:
                # positive-pair diagonal (stt is DVE-only on real HW)
                s = j - 16
                cols = slice(s * 128, (s + 1) * 128)
                junk = junkp.tile([128, 128], bf16)
                nc.vector.scalar_tensor_tensor(
                    out=junk[:], in0=E[:, cols], scalar=1.0,
                    in1=identb[:], op0=OP.mult, op1=OP.mult,
                    accum_out=posE[:, l, s:s + 1])

        def tail_l(l):
            den = denps[:, l * MS:(l + 1) * MS]
            logd = singles.tile([128, MS], f32)
            nc.scalar.activation(out=logd[:], in_=den, func=AF.Ln,
                                 bias=negclamp[:])
            posln = singles.tile([128, MS], f32)
            nc.scalar.activation(out=posln[:], in_=posE[:, l, :], func=AF.Ln)
            posT = singles.tile([128, MS], f32)
            nc.gpsimd.tensor_tensor(out=posT[:], in0=posln[:],
                                    in1=invs[l][:, 0:MS], op=OP.mult)
            lp = singles.tile([128, MS], f32)
            nc.gpsimd.tensor_tensor(out=lp[:], in0=logd[:], in1=posT[:],
                                    op=OP.subtract)
            nc.gpsimd.tensor_tensor(out=out_sb[:, l, :], in0=lp[:], in1=w[:],
                                    op=OP.mult)

        prologue(0)
        for l in range(L):
            inv = state[(l, "inv")]
            invT = state[(l, "invT")]
            An = state[(l, "An")]
            zT = state[(l, "zT")]
            invs[l] = inv
            for kind, j in SCHED:
                if l + 1 < L:
                    for pos, fn in PIPE:
                        if j == pos:
                            fn(l + 1)
                sim = simp.tile([128, R], f32)
                nc.tensor.matmul(sim[:], XTs[l][:, j * 128:(j + 1) * 128],
                                 zT[:], start=True, stop=True)
                if kind == "A":
                    E = ep.tile([128, R], bf16)
                    nc.scalar.activation(out=E[:], in_=sim[:], func=AF.Exp,
                                         scale=invT[:, j:j + 1])
                    extras(l, j, E)
                else:
                    E = ep.tile([128, R], i32)
                    nc.vector.tensor_scalar(
                        out=E[:], in0=sim[:], scalar1=An[:, j:j + 1],
                        scalar2=SCH_B, op0=OP.mult, op1=OP.add)
                pend.append((l, j, E, kind))
                if len(pend) > 7:
                    flush_colsum(pend.pop(0))
        while pend:
            flush_colsum(pend.pop(0))

        # ---- tail ----
        out_sb = singles.tile([128, L, MS], f32)
        for l in range(L):
            den = persist[:, 128 + l * MS:128 + (l + 1) * MS]
            logd = singles.tile([128, MS], f32)
            nc.scalar.activation(out=logd[:], in_=den, func=AF.Ln, bias=-CLAMP)
            posln = singles.tile([128, MS], f32)
            nc.scalar.activation(out=posln[:], in_=posE[:, l, :], func=AF.Ln)
            posT = singles.tile([128, MS], f32)
            nc.vector.tensor_tensor(out=posT[:], in0=posln[:],
                                    in1=invs[l][:, 0:MS], op=OP.mult)
            lp = singles.tile([128, MS], f32)
            nc.vector.tensor_tensor(out=lp[:], in0=logd[:], in1=posT[:],
                                    op=OP.subtract)
            nc.vector.tensor_tensor(out=out_sb[:, l, :], in0=lp[:], in1=w[:],
                                    op=OP.mult)
        nc.sync.dma_start(out=out[:, :], in_=out_sb[:].rearrange("p l s -> p (l s)"))

    nc.finalize()
    _built = nc
    return nc


def _in_maps(emb_i, emb_j, joint_valid):
    import ml_dtypes
    emb_i = np.asarray(emb_i, dtype=np.float32)
    emb_j = np.asarray(emb_j, dtype=np.float32)
    jv = np.asarray(joint_valid, dtype=np.float32).reshape(-1)
    reps = np.concatenate(
        [emb_i.reshape(L, N, D), emb_j.reshape(L, N, D)], axis=1)  # [L, M, D]
    maps = []
    for c in range(NCORES):
        idx = (np.arange(M) + c * R) % M
        rolled = reps[:, idx, :]                       # [L, M, D]
        xt = np.ascontiguousarray(
            rolled.transpose(2, 0, 1)).astype(ml_dtypes.bfloat16)
        xl = np.ascontiguousarray(
            rolled[:, :R, :].reshape(L, MS, 128, D).transpose(2, 0, 1, 3)
        ).astype(ml_dtypes.bfloat16)
        w = jv[(np.arange(R) + c * R) % N].reshape(MS, 128).T  # [128, MS]
        maps.append({"xt": xt, "xloc": xl, "wv": np.ascontiguousarray(w)})
    return maps, jv


def _combine(results, jv):
    tot = 0.0
    for r in results:
        tot += float(r["out_wlp"].astype(np.float64).sum())
    return np.float32(tot / (2.0 * float(jv.sum())))


def kernel(emb_i, emb_j, joint_valid):
    from concourse.bass_utils import run_bass_kernel_spmd

    nc = _build()
    maps, jv = _in_maps(emb_i, emb_j, joint_valid)
    res = run_bass_kernel_spmd(nc, maps, core_ids=list(range(NCORES)))
    return _combine(res.results, jv)


def run_traced(inputs, trace_cores=None):
    """test.py helper: same run but with NTFF tracing enabled."""
    from concourse.bass_utils import run_bass_kernel_spmd

    nc = _build()
    maps, jv = _in_maps(**inputs)
    res = run_bass_kernel_spmd(
        nc, maps, core_ids=list(range(NCORES)), trace=True,
        trace_cores=trace_cores if trace_cores is not None else list(range(NCORES)))
    res.loss = _combine(res.results, jv)
    return res
